# revision 1
# baseline (speedup 1.0000x reference)
"""Cross-attention kernel for Trainium2, 8 NeuronCores.

Problem: b=4, s=2048, d_model=1024, n_heads=16 (head_dim=64), fp32.
  out = softmax((q@Wq) (k@Wk)^T / sqrt(64) + mask) @ (v@Wv) @ Wo + bo

Sharding: core c handles batch c//2 and head-group c%2 (8 heads, 512
projection columns). Each core computes a partial output (s, 1024) =
(its heads' attention output) @ Wo[rows of its heads]; the host sums
the two partials per batch and adds bo.

Per-core algorithm (all matmuls fp32r = full PE rate for N>=256):
  Phase A: QhT (512, s) = Wq_c^T @ q^T   (head features on partitions)
           KhT (512, s) = Wk_c^T @ k^T
           Vh  (s, 520) = v @ Wv_c, stored head-interleaved with a
                column of ones appended per head (cols h*65+64). The
                ones come from the bias row; attention_mask multiplies
                Vh rows (masked j contributes 0 to both the numerator
                and the denominator of softmax).
  Phase B: per i-chunk (512 queries) per head:
           S^T tiles (128 j, 512 i) = KhT_h^T @ QhT_h  (PSUM)
           expS = exp(0.125 * S^T)                      (ACT, PSUM->SBUF)
           O^T (65, 512) += Vh_aug[jt]^T @ expS[jt]     (rows 0-63 =
                numerator^T, row 64 = softmax denominator)
           after all heads of the chunk: reciprocal of the 8 denom
           rows at once, GPSIMD partition-broadcast, multiply, stage
           the divided AttnOut^T chunk to DRAM.
  Phase C: Y (s, 1024) = AttnOut @ Wo_c, PSUM accumulated over the 4
           feature tiles, DMA'd to the output.
"""

import numpy as np

import concourse.bass as bass
import concourse.tile as tile
from concourse import mybir
from concourse.bass_utils import run_bass_kernel_spmd

P = 128
S = 2048          # sequence length
DIN = 1024        # model dim
C = 512           # projection columns per core (8 heads * 64)
NHC = 8           # heads per core
HD = 64           # head dim
VW = NHC * (HD + 1)   # 520: head-interleaved V width incl. ones columns
NIC = S // 512    # 4 i-chunks
NJT = S // P      # 16 j-tiles
F32 = mybir.dt.float32
F32R = mybir.dt.float32r


def _build_kernel():
    nc = bass.Bass("TRN2", target_bir_lowering=False, debug=False)

    qT = nc.dram_tensor("qT", [DIN, S], F32R, kind="ExternalInput").ap()
    kT = nc.dram_tensor("kT", [DIN, S], F32R, kind="ExternalInput").ap()
    vT = nc.dram_tensor("vT", [DIN, S], F32R, kind="ExternalInput").ap()
    wq = nc.dram_tensor("wq", [DIN, C], F32R, kind="ExternalInput").ap()
    wk = nc.dram_tensor("wk", [DIN, C], F32R, kind="ExternalInput").ap()
    wv = nc.dram_tensor("wv", [DIN, VW], F32R, kind="ExternalInput").ap()
    wo = nc.dram_tensor("wo", [C, DIN], F32R, kind="ExternalInput").ap()
    bq = nc.dram_tensor("bq", [C], F32, kind="ExternalInput").ap()
    bk = nc.dram_tensor("bk", [C], F32, kind="ExternalInput").ap()
    bva = nc.dram_tensor("bva", [VW], F32, kind="ExternalInput").ap()
    mm = nc.dram_tensor("mm", [S], F32, kind="ExternalInput").ap()
    y = nc.dram_tensor("y", [S, DIN], F32, kind="ExternalOutput").ap()

    ao_dram = nc.dram_tensor("ao_st", [P, 4, S], F32R).ap()  # AttnOut^T staging
    rcp_dram = nc.dram_tensor("rcp_st", [32, 512], F32).ap()  # 1/denom bounce

    with tile.TileContext(nc) as tc:
        _body(tc, y, ao_dram, rcp_dram, qT, kT, vT, wq, wk, wv, wo, bq, bk,
              bva, mm)
    return nc


def _bcast_rows(ap, parts):
    """AP reading a 1-D DRAM tensor broadcast over `parts` partitions."""
    return bass.AP(tensor=ap.tensor, offset=ap.offset, ap=[[0, parts]] + list(ap.ap))


def _body(tc, y, ao_dram, rcp_dram, qT, kT, vT, wq, wk, wv, wo, bq, bk, bva,
          mm):
    nc = tc.nc

    with tc.tile_pool(name="qkv", bufs=1) as qkv_pool:
        qhT = qkv_pool.tile([P, 4, S], F32R)    # [dout%128, dout//128, i]
        khT = qkv_pool.tile([P, 4, S], F32R)
        vh = qkv_pool.tile([P, NJT, VW], F32R)  # [j%128, j//128, 8*(64+1)]

        # ---------------- Phase A: projections ----------------
        with (
            tc.tile_pool(name="wpool", bufs=1) as wpool,
            tc.tile_pool(name="xin", bufs=1) as xin,
        ):
            wq_sb = wpool.tile([P, 8, C], F32R)
            nc.sync.dma_start(out=wq_sb, in_=wq.rearrange("(t p) c -> p t c", p=P))
            wk_sb = wpool.tile([P, 8, C], F32R)
            nc.sync.dma_start(out=wk_sb, in_=wk.rearrange("(t p) c -> p t c", p=P))
            wv_sb = wpool.tile([P, 8, VW], F32R)
            nc.sync.dma_start(out=wv_sb, in_=wv.rearrange("(t p) c -> p t c", p=P))
            bq_sb = wpool.tile([P, 4], F32)
            nc.sync.dma_start(out=bq_sb, in_=bq.rearrange("(t p) -> p t", p=P))
            bk_sb = wpool.tile([P, 4], F32)
            nc.sync.dma_start(out=bk_sb, in_=bk.rearrange("(t p) -> p t", p=P))
            bvb = wpool.tile([P, VW], F32)
            nc.sync.dma_start(out=bvb, in_=_bcast_rows(bva, P))
            mm_sb = wpool.tile([P, NJT], F32)
            nc.sync.dma_start(out=mm_sb, in_=mm.rearrange("(t p) -> p t", p=P))

            # QhT / KhT: dst[do, i] = sum_k W[k, do] * xT[k, i]
            for x_dram, w_sb, b_sb, dst, nm in (
                (qT, wq_sb, bq_sb, qhT, "q"),
                (kT, wk_sb, bk_sb, khT, "k"),
            ):
                with tc.tile_pool(name=f"ps{nm}", bufs=4, space="PSUM") as psp:
                    for ic in range(NIC):
                        xts = []
                        for kt in range(8):
                            xt = xin.tile([P, 512], F32R, tag=f"x{kt}")
                            nc.sync.dma_start(
                                out=xt,
                                in_=x_dram[kt * P:(kt + 1) * P,
                                           ic * 512:(ic + 1) * 512],
                            )
                            xts.append(xt)
                        for io in range(4):
                            ps = psp.tile([P, 512], F32)
                            for kt in range(8):
                                nc.tensor.matmul(
                                    ps,
                                    (w_sb[:, kt, io * P:(io + 1) * P]),
                                    (xts[kt]),
                                    start=(kt == 0),
                                    stop=(kt == 7),
                                )
                            nc.vector.tensor_scalar_add(
                                out=dst[:, io, ic * 512:(ic + 1) * 512],
                                in0=ps,
                                scalar1=b_sb[:, io:io + 1],
                            )

            # Vh: dst[j, c] = sum_k vT[k, j] * Wv[k, c]; + bias row; * mask
            with tc.tile_pool(name="psv", bufs=3, space="PSUM") as psv:
                for jg in range(4):           # groups of 4 j-tiles
                    xts = []
                    for kt in range(8):
                        xt = xin.tile([P, 512], F32R, tag=f"x{kt}")
                        nc.sync.dma_start(
                            out=xt,
                            in_=vT[kt * P:(kt + 1) * P,
                                   jg * 512:(jg + 1) * 512],
                        )
                        xts.append(xt)
                    for ji in range(4):
                        jt = jg * 4 + ji
                        ps = psv.tile([P, VW], F32)
                        for kt in range(8):
                            nc.tensor.matmul(
                                ps[:, 0:512],
                                (xts[kt][:, ji * P:(ji + 1) * P]),
                                (wv_sb[:, kt, 0:512]),
                                start=(kt == 0),
                                stop=(kt == 7),
                            )
                        for kt in range(8):
                            nc.tensor.matmul(
                                ps[:, 512:VW],
                                (xts[kt][:, ji * P:(ji + 1) * P]),
                                (wv_sb[:, kt, 512:VW]),
                                start=(kt == 0),
                                stop=(kt == 7),
                            )
                        nc.vector.tensor_tensor(
                            out=vh[:, jt, :],
                            in0=ps,
                            in1=bvb,
                            op=mybir.AluOpType.add,
                        )
                        nc.vector.tensor_scalar_mul(
                            out=vh[:, jt, :],
                            in0=vh[:, jt, :],
                            scalar1=mm_sb[:, jt:jt + 1],
                        )

        # ---------------- Phase B: attention ----------------
        with (
            tc.tile_pool(name="st", bufs=3, space="PSUM") as stp,
            tc.tile_pool(name="ot", bufs=2, space="PSUM") as otp,
            tc.tile_pool(name="ex", bufs=3) as exp_pool,
            tc.tile_pool(name="stage", bufs=2) as stage_pool,
            tc.tile_pool(name="divp", bufs=2) as divp,
        ):
            for ic in range(NIC):
                stg = stage_pool.tile([P, 4, 512], F32R)
                for h in range(NHC):
                    hp = (h % 2) * HD          # partition offset of head rows
                    hb = h // 2                # 128-block of head rows
                    ot = otp.tile([HD + 1, 512], F32)
                    for jp in range(NJT // 2):
                        st = stp.tile([P, 1024], F32)
                        for u in range(2):
                            jt = jp * 2 + u
                            nc.tensor.matmul(
                                st[:, u * 512:(u + 1) * 512],
                                (khT[hp:hp + HD, hb, jt * P:(jt + 1) * P]),
                                (qhT[hp:hp + HD, hb, ic * 512:(ic + 1) * 512]),
                                start=True,
                                stop=True,
                            )
                        ex = exp_pool.tile([P, 1024], F32R)
                        nc.scalar.activation(
                            out=ex,
                            in_=st,
                            func=mybir.ActivationFunctionType.Exp,
                            scale=float(HD) ** -0.5,
                        )
                        for u in range(2):
                            jt = jp * 2 + u
                            nc.tensor.matmul(
                                ot,
                                (vh[:, jt, h * (HD + 1):(h + 1) * (HD + 1)]),
                                (ex[:, u * 512:(u + 1) * 512]),
                                start=(jt == 0),
                                stop=(jt == NJT - 1),
                            )
                    # divide numerator^T rows by the denominator row and
                    # move the result into the staging tile
                    rcp = divp.tile([1, 512], F32, tag="rcp")
                    nc.vector.reciprocal(out=rcp, in_=ot[HD:HD + 1, :])
                    slot = ic * 8 + h
                    nc.sync.dma_start(
                        out=rcp_dram[slot:slot + 1, :], in_=rcp
                    )
                    bc = divp.tile([HD, 512], F32, tag="bc")
                    row = rcp_dram[slot:slot + 1, :]
                    nc.sync.dma_start(
                        out=bc,
                        in_=bass.AP(tensor=row.tensor, offset=row.offset,
                                    ap=[[0, HD]] + list(row.ap)[1:]),
                    )
                    nc.vector.tensor_tensor(
                        out=stg[hp:hp + HD, hb, :],
                        in0=ot[0:HD, :],
                        in1=bc,
                        op=mybir.AluOpType.mult,
                    )
                nc.sync.dma_start(
                    out=ao_dram[:, :, ic * 512:(ic + 1) * 512], in_=stg
                )

    # ---------------- Phase C: output projection ----------------
    with (
        tc.tile_pool(name="cpool", bufs=1) as cpool,
        tc.tile_pool(name="ysb", bufs=3) as ysb_pool,
        tc.tile_pool(name="psy", bufs=4, space="PSUM") as psy,
    ):
        wo_sb = cpool.tile([P, 4, DIN], F32R)
        nc.sync.dma_start(out=wo_sb, in_=wo.rearrange("(t p) c -> p t c", p=P))
        ao_sb = cpool.tile([P, 4, S], F32R)
        nc.sync.dma_start(out=ao_sb, in_=ao_dram)
        for it in range(S // P):
            for ec in range(2):
                ps = psy.tile([P, 512], F32)
                for ft in range(4):
                    nc.tensor.matmul(
                        ps,
                        (ao_sb[:, ft, it * P:(it + 1) * P]),
                        (wo_sb[:, ft, ec * 512:(ec + 1) * 512]),
                        start=(ft == 0),
                        stop=(ft == 3),
                    )
                yt = ysb_pool.tile([P, 512], F32)
                nc.vector.tensor_copy(out=yt, in_=ps)
                nc.sync.dma_start(
                    out=y[it * P:(it + 1) * P, ec * 512:(ec + 1) * 512], in_=yt
                )


def _legalize_sync(bir, max_waits=1, max_updates=1):
    """Split sync lists so every instruction carries at most `max_waits`
    waits and `max_updates` updates; the walrus build in this container
    rejects instructions with more ("Too many sync wait commands").
    Extra waits go on EventSemaphore instructions inserted just before
    (same engine => same program order), extra updates just after."""
    n = [0]

    def ev(engine, debug, waits, updates):
        n[0] += 1
        return {
            "debug": debug,
            "engine": engine,
            "ins": [],
            "outs": [],
            "name": f"I-syncsplit-{n[0]}",
            "opcode": "EventSemaphore",
            "sync_info": {"on_wait": waits, "on_update": updates},
        }

    for fn in bir["functions"]:
        for bb in fn["blocks"]:
            out = []
            for ins in bb["instructions"]:
                si = ins.get("sync_info")
                eng = ins.get("engine")
                post = []
                if si and eng:
                    waits = si.get("on_wait") or []
                    updates = si.get("on_update") or []
                    dbg = ins.get("debug", 0)
                    while len(waits) > max_waits:
                        chunk, waits = waits[:max_waits], waits[max_waits:]
                        out.append(ev(eng, dbg, chunk, []))
                    while len(updates) > max_updates:
                        updates, chunk = updates[:-max_updates], updates[-max_updates:]
                        post.append(ev(eng, dbg, [], chunk))
                    si["on_wait"] = waits
                    si["on_update"] = updates
                out.append(ins)
                out.extend(reversed(post))
            bb["instructions"] = out


_NC_CACHE = {}


def _get_nc():
    if "nc" not in _NC_CACHE:
        import json as _json

        nc = _build_kernel()
        orig = nc.to_json_bytes

        def patched():
            bir = _json.loads(orig())
            _legalize_sync(bir)
            return _json.dumps(bir).encode()

        nc.to_json_bytes = patched
        _NC_CACHE["nc"] = nc
    return _NC_CACHE["nc"]


def make_in_maps(q, k, v, attention_mask, Wq, bq, Wk, bk, Wv, bv, Wo, bo):
    """Host-side sharding: returns the per-core input maps."""
    q = np.asarray(q, np.float32)
    k = np.asarray(k, np.float32)
    v = np.asarray(v, np.float32)
    Wq = np.asarray(Wq, np.float32)
    Wk = np.asarray(Wk, np.float32)
    Wv = np.asarray(Wv, np.float32)
    Wo = np.asarray(Wo, np.float32)
    bq = np.asarray(bq, np.float32)
    bk = np.asarray(bk, np.float32)
    bv = np.asarray(bv, np.float32)
    mask = np.asarray(attention_mask)

    in_maps = []
    for c in range(8):
        bc, hg = c // 2, c % 2
        cs = slice(hg * C, (hg + 1) * C)
        wv_aug = np.zeros((DIN, VW), np.float32)
        bv_aug = np.zeros((VW,), np.float32)
        for h in range(NHC):
            src = slice(hg * C + h * HD, hg * C + (h + 1) * HD)
            dst = slice(h * (HD + 1), h * (HD + 1) + HD)
            wv_aug[:, dst] = Wv[:, src]
            bv_aug[dst] = bv[src]
            bv_aug[h * (HD + 1) + HD] = 1.0
        in_maps.append({
            "qT": np.ascontiguousarray(q[bc].T),
            "kT": np.ascontiguousarray(k[bc].T),
            "vT": np.ascontiguousarray(v[bc].T),
            "wq": np.ascontiguousarray(Wq[:, cs]),
            "wk": np.ascontiguousarray(Wk[:, cs]),
            "wv": wv_aug,
            "wo": np.ascontiguousarray(Wo[cs, :]),
            "bq": np.ascontiguousarray(bq[cs]),
            "bk": np.ascontiguousarray(bk[cs]),
            "bva": bv_aug,
            "mm": mask[bc].astype(np.float32),
        })
    return in_maps


def kernel(q, k, v, attention_mask, Wq, bq, Wk, bk, Wv, bv, Wo, bo, _trace=False):
    in_maps = make_in_maps(
        q, k, v, attention_mask, Wq, bq, Wk, bk, Wv, bv, Wo, bo
    )
    nc = _get_nc()
    import time as _time
    t0 = _time.time()
    try:
        res = run_bass_kernel_spmd(nc, in_maps, list(range(8)), trace=_trace)
    except Exception:
        if not _trace:
            raise
        res = run_bass_kernel_spmd(nc, in_maps, list(range(8)))
    kernel._last_run_seconds = _time.time() - t0
    bo = np.asarray(bo, np.float32)
    out = np.stack(
        [res.results[2 * b]["y"] + res.results[2 * b + 1]["y"] + bo
         for b in range(4)]
    ).astype(np.float32)
    if _trace:
        kernel._last_results = res
    return out



# revision 22
# speedup vs baseline: 3.9717x; 3.9717x over previous
"""Cross-attention kernel for Trainium2, 8 NeuronCores.

Problem: b=4, s=2048, d_model=1024, n_heads=16 (head_dim=64), fp32.
  out = softmax((q@Wq) (k@Wk)^T / sqrt(64) + mask) @ (v@Wv) @ Wo + bo

Sharding: core c handles batch c//2 and head-group c%2 (8 heads, 512
projection columns). Each core computes a partial output (s, 1024) =
(its heads' attention output) @ Wo[rows of its heads]; the host sums
the two partials per batch and adds bo.

v2 design (single fused pipeline, ACT-engine bound):
  * All matmul operands bf16 (inputs converted host-side); PSUM fp32.
  * S^T tiles for a HEAD PAIR computed concurrently via PE row tiling
    (contraction=64 each: head A rows 0-63, head B rows 64-127), into
    one [128, 1024] PSUM pair tile -> one exp() per j-tile.
  * AV keeps the ones-column trick (M=65: numerator^T rows + softmax
    denominator row); V stored head-interleaved with mask-scaled ones
    columns written by DVE (no N=8 tail matmuls).
  * K/V/Q projections pipelined INTO the attention stream (j-group
    granularity for ic=0; Q(ic+1) and O-proj(ic) groups interleaved as
    PE filler), AttnOut staged in SBUF (no DRAM bounce), so the scalar
    engine (exp: 33.5M elem/core ~ 220us floor) is saturated end2end.
  * Softmax division: ot PSUM copied to SBUF immediately (frees the
    single-buffered ot banks), reciprocal of the denominator row is
    partition-broadcast via a DRAM bounce, multiply on DVE.
PSUM: st pair tiles 2x[128,1024] (4 banks) + ot A/B 2x[128,512]
(2 banks, single-buffered) + proj/O-proj ring 2x[128,512] (2 banks).
"""

import collections

import numpy as np

import concourse.bass as bass
import concourse.tile as tile
from concourse import mybir
from concourse.bass_utils import run_bass_kernel_spmd

P = 128
S = 2048          # sequence length
DIN = 1024        # model dim
C = 512           # projection columns per core (8 heads * 64)
NHC = 8           # heads per core
HD = 64           # head dim
VW = NHC * (HD + 1)   # 520: head-interleaved V width incl. ones columns
NIC = S // 512    # 4 i-chunks
NJT = S // P      # 16 j-tiles
LAG = 2           # S^T -> AV software-pipeline distance (j-tiles)
F32 = mybir.dt.float32
BF16 = mybir.dt.bfloat16


def _build_kernel():
    nc = bass.Bass("TRN2", target_bir_lowering=False, debug=False)

    qT = nc.dram_tensor("qT", [DIN, S], BF16, kind="ExternalInput").ap()
    kT = nc.dram_tensor("kT", [DIN, S], BF16, kind="ExternalInput").ap()
    vT = nc.dram_tensor("vT", [DIN, S], BF16, kind="ExternalInput").ap()
    wq = nc.dram_tensor("wq", [DIN, C], BF16, kind="ExternalInput").ap()
    wk = nc.dram_tensor("wk", [DIN, C], BF16, kind="ExternalInput").ap()
    wv = nc.dram_tensor("wv", [DIN, C], BF16, kind="ExternalInput").ap()
    wo = nc.dram_tensor("wo", [C, DIN], BF16, kind="ExternalInput").ap()
    bq = nc.dram_tensor("bq", [C], F32, kind="ExternalInput").ap()
    bk = nc.dram_tensor("bk", [C], F32, kind="ExternalInput").ap()
    bv = nc.dram_tensor("bv", [C], F32, kind="ExternalInput").ap()
    mm = nc.dram_tensor("mm", [S], F32, kind="ExternalInput").ap()
    onec = nc.dram_tensor("onec", [NHC], F32, kind="ExternalInput").ap()
    y = nc.dram_tensor("y", [S, DIN], F32, kind="ExternalOutput").ap()

    rcp_dram = nc.dram_tensor("rcp_st", [32, 512], F32).ap()  # 1/denom bounce

    with tile.TileContext(nc) as tc:
        _body(tc, y, rcp_dram, qT, kT, vT, wq, wk, wv, wo, bq, bk, bv, mm,
              onec)
    return nc


def _bcast_rows(ap, parts):
    """AP reading a 1-D (or row) DRAM region broadcast over `parts`
    partitions."""
    return bass.AP(tensor=ap.tensor, offset=ap.offset,
                   ap=[[0, parts]] + list(ap.ap))


def _blocks(ap2d, nblk, blk_stride, width, blk_off=0):
    """[p, nblk, width] AP over a 2-D [p, F] slice: blocks of `width`
    elements every `blk_stride`, starting at `blk_off`."""
    return bass.AP(tensor=ap2d.tensor, offset=ap2d.offset + blk_off,
                   ap=[list(ap2d.ap[0]), [blk_stride, nblk], [1, width]])


def _strided(ap2d, start, stride, count):
    """[p, count] AP: one element every `stride`, starting at `start`."""
    return bass.AP(tensor=ap2d.tensor, offset=ap2d.offset + start,
                   ap=[list(ap2d.ap[0]), [stride, count]])


def _xsrc(x_dram, col0, cols):
    """[128, 8, cols] AP over x_dram [1024, S]: partition = row%128,
    block = row//128, innermost = cols starting at col0."""
    return bass.AP(tensor=x_dram.tensor, offset=x_dram.offset + col0,
                   ap=[[S, P], [P * S, 8], [1, cols]])


def _body(tc, y, rcp_dram, qT, kT, vT, wq, wk, wv, wo, bq, bk, bv, mm, onec):
    nc = tc.nc

    with (
        tc.tile_pool(name="wpool", bufs=1) as wpool,
        tc.tile_pool(name="big", bufs=1) as big,
        tc.tile_pool(name="xin", bufs=2) as xin,
        tc.tile_pool(name="expool", bufs=8) as expool,
        tc.tile_pool(name="cppool", bufs=2) as cppool,
        tc.tile_pool(name="bcpool", bufs=2) as bcpool,
        tc.tile_pool(name="rcpp", bufs=2) as rcpp,
        tc.tile_pool(name="ytp", bufs=2) as ytp,
        tc.tile_pool(name="stp", bufs=2, space="PSUM") as stp,
        tc.tile_pool(name="otp", bufs=1, space="PSUM") as otp,
        tc.tile_pool(name="pjp", bufs=2, space="PSUM") as pjp,
    ):
        # ---------------- static tiles + weight DMAs ----------------
        # DMA transfers serialize in emission order (single-queue FIFO in
        # the model; shared HBM BW on hw), so emission order = arrival
        # priority: K-projection operands (wk+xk) gate the very first
        # matmuls, then Q, then V; wo can land tens of microseconds in.
        wk_sb = wpool.tile([P, 8, C], BF16)
        nc.sync.dma_start(out=wk_sb, in_=wk.rearrange("(t p) c -> p t c", p=P))

        qhT = big.tile([P, 4, S], BF16)   # [dout%128, dout//128, i]
        khT = big.tile([P, 4, S], BF16)   # [dout%128, dout//128, j]
        vh = big.tile([P, NJT, VW], BF16)  # [j%128, j//128, 8*(64+1)]
        stg = big.tile([P, 4, S], BF16)   # AttnOut^T [feat%128, pair, i]

        # ---------------- helper closures ----------------
        def dma_x(tag, x_dram, col0, cols=512):
            xt = xin.tile([P, 8, cols], BF16, tag=tag, name=f"x{tag}")
            nc.sync.dma_start(out=xt, in_=_xsrc(x_dram, col0, cols))
            return xt

        def qk_group(dst, w_sb, b_sb, xt, io, c0):
            """dst[:, io, c0:c0+512] = (W[:, io-block]^T @ x) + bias"""
            ps = pjp.tile([P, 512], F32, name="pspj")
            for kt in range(8):
                nc.tensor.matmul(
                    ps,
                    (w_sb[:, kt, io * P:(io + 1) * P]),
                    (xt[:, kt, :]),
                    start=(kt == 0),
                    stop=(kt == 7),
                )
            nc.vector.tensor_scalar_add(
                out=dst[:, io, c0:c0 + 512],
                in0=ps,
                scalar1=b_sb[:, io:io + 1],
            )

        def v_group(jt, xt):
            """vh[:, jt, :] = interleave((x_jt @ Wv) + bv, ones) * mask"""
            ji = jt % 4
            ps = pjp.tile([P, 512], F32, name="pspj")
            for kt in range(8):
                nc.tensor.matmul(
                    ps,
                    (xt[:, kt, ji * P:(ji + 1) * P]),
                    (wv_sb[:, kt, :]),
                    start=(kt == 0),
                    stop=(kt == 7),
                )
            v2d = vh[:, jt, :]
            numer = _blocks(v2d, NHC, HD + 1, HD)
            nc.vector.tensor_tensor(
                out=numer,
                in0=_blocks(ps, NHC, HD, HD),
                in1=_blocks(bvb, NHC, HD, HD),
                op=mybir.AluOpType.add,
            )
            nc.vector.tensor_scalar_mul(
                out=numer,
                in0=numer,
                scalar1=mm_sb[:, jt:jt + 1],
            )
            nc.vector.tensor_scalar_mul(
                out=_strided(v2d, HD, HD + 1, NHC),
                in0=ones8,
                scalar1=mm_sb[:, jt:jt + 1],
            )

        def st_step(ic, p, jt):
            """S^T pair tile for heads (2p, 2p+1) at (ic, jt) + exp."""
            st = stp.tile([P, 1024], F32, name="st")
            for u in range(2):
                nc.tensor.matmul(
                    st[:, u * 512:(u + 1) * 512],
                    (khT[u * HD:(u + 1) * HD, p, jt * P:(jt + 1) * P]),
                    (qhT[u * HD:(u + 1) * HD, p, ic * 512:(ic + 1) * 512]),
                    start=True,
                    stop=True,
                )
            ex = expool.tile([P, 1024], BF16, name="ex")
            nc.scalar.activation(
                out=ex,
                in_=st,
                func=mybir.ActivationFunctionType.Exp,
                scale=float(HD) ** -0.5,
            )
            return ex

        def av_step(ex, otA, otB, p, jt):
            for u, ot in ((0, otA), (1, otB)):
                h = 2 * p + u
                nc.tensor.matmul(
                    ot[0:HD + 1, :],
                    (vh[:, jt, h * (HD + 1):(h + 1) * (HD + 1)]),
                    (ex[:, u * 512:(u + 1) * 512]),
                    start=(jt == 0),
                    stop=(jt == NJT - 1),
                )

        def division(ic, p, otA, otB, last=False):
            """stg[:, p, ic-block] = numerators / denominator.

            Normally the full-tile copies run first so the single-buffered
            ot banks free ASAP (they gate the next pair's AV matmuls); on
            the last pair nothing follows, so the reciprocals read PSUM
            directly to start the DMA broadcast bounce sooner."""
            r2 = rcpp.tile([1, 1024], F32, name="r2")
            cA = cppool.tile([HD + 1, 512], F32, tag="cA", name="cA")
            cB = cppool.tile([HD + 1, 512], F32, tag="cB", name="cB")
            if last:
                nc.vector.reciprocal(out=r2[:, 0:512], in_=otA[HD:HD + 1, :])
                nc.vector.reciprocal(out=r2[:, 512:1024], in_=otB[HD:HD + 1, :])
                nc.vector.tensor_copy(out=cA, in_=otA[0:HD + 1, :])
                nc.vector.tensor_copy(out=cB, in_=otB[0:HD + 1, :])
            else:
                nc.vector.tensor_copy(out=cA, in_=otA[0:HD + 1, :])
                nc.vector.tensor_copy(out=cB, in_=otB[0:HD + 1, :])
                nc.vector.reciprocal(out=r2[:, 0:512], in_=cA[HD:HD + 1, :])
                nc.vector.reciprocal(out=r2[:, 512:1024], in_=cB[HD:HD + 1, :])
            slot = 2 * (ic * 4 + p)
            rows = rcp_dram[slot:slot + 2, :]
            # SWDGE (gpsimd-issued) DMAs: the sem-wait on the reciprocals
            # would otherwise block the SP DMA-issue FIFO, delaying every
            # bulk transfer queued behind it.
            nc.gpsimd.dma_start(out=rows, in_=r2)
            bc = bcpool.tile([HD, 1024], F32, name="bc")
            # one DMA: each denominator row broadcast over 64 partitions
            # (A's in cols 0-511, B's in 512-1023, both at base partition 0)
            nc.gpsimd.dma_start(
                out=bc,
                in_=bass.AP(tensor=rows.tensor, offset=rows.offset,
                            ap=[[0, HD], [512, 2], [1, 512]]))
            icb = slice(ic * 512, (ic + 1) * 512)
            # one multiply on DVE, one on the otherwise-idle Pool engine:
            # they run in parallel, halving the division tail.
            nc.vector.tensor_tensor(
                out=stg[0:HD, p, icb], in0=cA[0:HD, :], in1=bc[:, 0:512],
                op=mybir.AluOpType.mult)
            nc.gpsimd.tensor_tensor(
                out=stg[HD:P, p, icb], in0=cB[0:HD, :], in1=bc[:, 512:1024],
                op=mybir.AluOpType.mult)

        def o_group(it, ec, act=False):
            """y[it-block, ec-block] = AttnOut[it] @ Wo[:, ec-block].
            act=True drains via the scalar engine (idle in the epilogue;
            Copy is in every activation table set so no table reload) and
            borrows the by-then-idle st PSUM ring for deeper pipelining."""
            if act:
                ps = stp.tile([P, 512], F32, name="pso")
            else:
                ps = pjp.tile([P, 512], F32, name="pspj")
            for ft in range(4):
                nc.tensor.matmul(
                    ps,
                    (stg[:, ft, it * P:(it + 1) * P]),
                    (wo_sb[:, ft, ec * 512:(ec + 1) * 512]),
                    start=(ft == 0),
                    stop=(ft == 3),
                )
            yt = ytp.tile([P, 512], F32, name="yt")
            if act:
                nc.scalar.activation(
                    out=yt, in_=ps, func=mybir.ActivationFunctionType.Copy)
            else:
                nc.vector.tensor_copy(out=yt, in_=ps)
            nc.sync.dma_start(
                out=y[it * P:(it + 1) * P, ec * 512:(ec + 1) * 512], in_=yt)

        # ---------------- prologue ----------------
        xk = dma_x("xk", kT, 0)
        bk_sb = wpool.tile([P, 4], F32)
        nc.sync.dma_start(out=bk_sb, in_=bk.rearrange("(t p) -> p t", p=P))
        wq_sb = wpool.tile([P, 8, C], BF16)
        nc.sync.dma_start(out=wq_sb, in_=wq.rearrange("(t p) c -> p t c", p=P))
        xq = dma_x("xq", qT, 0)
        bq_sb = wpool.tile([P, 4], F32)
        nc.sync.dma_start(out=bq_sb, in_=bq.rearrange("(t p) -> p t", p=P))
        wv_sb = wpool.tile([P, 8, C], BF16)
        nc.sync.dma_start(out=wv_sb, in_=wv.rearrange("(t p) c -> p t c", p=P))
        xv = dma_x("xv", vT, 0)
        bvb = wpool.tile([P, C], F32)
        nc.sync.dma_start(out=bvb, in_=_bcast_rows(bv, P))
        mm_sb = wpool.tile([P, NJT], F32)
        nc.sync.dma_start(out=mm_sb, in_=mm.rearrange("(t p) -> p t", p=P))
        ones8 = wpool.tile([P, NHC], F32)
        nc.sync.dma_start(out=ones8, in_=_bcast_rows(onec, P))
        wo_sb = wpool.tile([P, 4, DIN], BF16)
        nc.sync.dma_start(out=wo_sb, in_=wo.rearrange("(t p) c -> p t c", p=P))
        for io in range(4):
            qk_group(khT, wk_sb, bk_sb, xk, io, 0)
        for io in range(4):
            qk_group(qhT, wq_sb, bq_sb, xq, io, 0)
        for jt in range(4):
            v_group(jt, xv)

        # ---------------- fused attention pipeline ----------------
        fillers = collections.deque()
        # ic0/pair0 K+V projection stream, consumed 2 per step (8 groups
        # per j-group of 4 steps -> each j-group ready exactly in time).
        kvx = [None, None, None, None]

        def mk_kv_dma(jg):
            def f():
                kvx[jg] = (dma_x("xk", kT, jg * 512), dma_x("xv", vT, jg * 512))
            return f

        def mk_k(jg, io):
            return lambda: qk_group(khT, wk_sb, bk_sb, kvx[jg][0], io, jg * 512)

        def mk_v(jg, jt):
            return lambda: v_group(jt, kvx[jg][1])

        ic0_fill = collections.deque()
        for jg in range(1, 4):
            ic0_fill.append(mk_kv_dma(jg))
            for io in range(4):
                ic0_fill.append(mk_k(jg, io))
            for jt in range(jg * 4, jg * 4 + 4):
                ic0_fill.append(mk_v(jg, jt))

        qx = [None]

        def mk_q_dma(ic):
            def f():
                qx[0] = dma_x("xq", qT, ic * 512)
            return f

        def mk_q(ic, io):
            return lambda: qk_group(qhT, wq_sb, bq_sb, qx[0], io, ic * 512)

        def mk_o(it, ec, act=False):
            return lambda: o_group(it, ec, act)

        pending = collections.deque()  # (ex, otA, otB, p, jt, ic)
        step = [0]

        def flush_one():
            ex, otA, otB, p, jt, pic = pending.popleft()
            av_step(ex, otA, otB, p, jt)
            if jt == NJT - 1:
                last = pic == NIC - 1 and p == 3
                division(pic, p, otA, otB, last=last)
                if p == 3:
                    for n, (it, ec) in enumerate(
                            (it, ec) for it in range(pic * 4, pic * 4 + 4)
                            for ec in range(2)):
                        fillers.append(mk_o(it, ec, act=last and n % 2 == 0))

        for ic in range(NIC):
            if ic < NIC - 1:
                fillers.append(mk_q_dma(ic + 1))
                for io in range(4):
                    fillers.append(mk_q(ic + 1, io))
            for p in range(4):
                otA = otp.tile([P, 512], F32, tag="A", name="otA")
                otB = otp.tile([P, 512], F32, tag="B", name="otB")
                for jt in range(NJT):
                    ex = st_step(ic, p, jt)
                    pending.append((ex, otA, otB, p, jt, ic))
                    if ic == 0 and p == 0:
                        for _ in range(2):
                            if ic0_fill:
                                ic0_fill.popleft()()
                    elif step[0] % 2 == 0 and fillers:
                        fillers.popleft()()
                    step[0] += 1
                    # hold a new pair's first AVs a few extra steps so the
                    # single-buffered ot banks' drain (DVE copy of the
                    # previous pair) hides behind S^T work
                    for _ in range(2):
                        need = LAG + 3 if pending[0][4] == 0 else LAG
                        if len(pending) <= need:
                            break
                        flush_one()

        while pending:
            flush_one()
        while fillers:
            fillers.popleft()()


def _legalize_sync(bir, max_waits=1, max_updates=1):
    """Split sync lists so every instruction carries at most `max_waits`
    waits and `max_updates` updates; the walrus build in this container
    rejects instructions with more ("Too many sync wait commands").
    Extra waits go on EventSemaphore instructions inserted just before
    (same engine => same program order), extra updates just after."""
    n = [0]

    def ev(engine, debug, waits, updates):
        n[0] += 1
        return {
            "debug": debug,
            "engine": engine,
            "ins": [],
            "outs": [],
            "name": f"I-syncsplit-{n[0]}",
            "opcode": "EventSemaphore",
            "sync_info": {"on_wait": waits, "on_update": updates},
        }

    for fn in bir["functions"]:
        for bb in fn["blocks"]:
            out = []
            for ins in bb["instructions"]:
                si = ins.get("sync_info")
                eng = ins.get("engine")
                post = []
                if si and eng:
                    waits = si.get("on_wait") or []
                    updates = si.get("on_update") or []
                    dbg = ins.get("debug", 0)
                    while len(waits) > max_waits:
                        chunk, waits = waits[:max_waits], waits[max_waits:]
                        out.append(ev(eng, dbg, chunk, []))
                    while len(updates) > max_updates:
                        updates, chunk = updates[:-max_updates], updates[-max_updates:]
                        post.append(ev(eng, dbg, [], chunk))
                    si["on_wait"] = waits
                    si["on_update"] = updates
                out.append(ins)
                out.extend(reversed(post))
            bb["instructions"] = out


_NC_CACHE = {}


def _get_nc():
    if "nc" not in _NC_CACHE:
        import json as _json

        nc = _build_kernel()
        orig = nc.to_json_bytes

        def patched():
            bir = _json.loads(orig())
            _legalize_sync(bir)
            return _json.dumps(bir).encode()

        nc.to_json_bytes = patched
        _NC_CACHE["nc"] = nc
    return _NC_CACHE["nc"]


def make_in_maps(q, k, v, attention_mask, Wq, bq, Wk, bk, Wv, bv, Wo, bo):
    """Host-side sharding: returns the per-core input maps."""
    import ml_dtypes

    bf = ml_dtypes.bfloat16
    q = np.asarray(q, np.float32)
    k = np.asarray(k, np.float32)
    v = np.asarray(v, np.float32)
    Wq = np.asarray(Wq, np.float32)
    Wk = np.asarray(Wk, np.float32)
    Wv = np.asarray(Wv, np.float32)
    Wo = np.asarray(Wo, np.float32)
    bq = np.asarray(bq, np.float32)
    bk = np.asarray(bk, np.float32)
    bv = np.asarray(bv, np.float32)
    mask = np.asarray(attention_mask)

    qTb = [np.ascontiguousarray(q[b].T).astype(bf) for b in range(4)]
    kTb = [np.ascontiguousarray(k[b].T).astype(bf) for b in range(4)]
    vTb = [np.ascontiguousarray(v[b].T).astype(bf) for b in range(4)]
    onec = np.ones((NHC,), np.float32)

    in_maps = []
    for c in range(8):
        bc, hg = c // 2, c % 2
        cs = slice(hg * C, (hg + 1) * C)
        in_maps.append({
            "qT": qTb[bc],
            "kT": kTb[bc],
            "vT": vTb[bc],
            "wq": np.ascontiguousarray(Wq[:, cs]).astype(bf),
            "wk": np.ascontiguousarray(Wk[:, cs]).astype(bf),
            "wv": np.ascontiguousarray(Wv[:, cs]).astype(bf),
            "wo": np.ascontiguousarray(Wo[cs, :]).astype(bf),
            "bq": np.ascontiguousarray(bq[cs]),
            "bk": np.ascontiguousarray(bk[cs]),
            "bv": np.ascontiguousarray(bv[cs]),
            "mm": mask[bc].astype(np.float32),
            "onec": onec,
        })
    return in_maps


def kernel(q, k, v, attention_mask, Wq, bq, Wk, bk, Wv, bv, Wo, bo, _trace=False):
    in_maps = make_in_maps(
        q, k, v, attention_mask, Wq, bq, Wk, bk, Wv, bv, Wo, bo
    )
    nc = _get_nc()
    import time as _time
    t0 = _time.time()
    try:
        res = run_bass_kernel_spmd(nc, in_maps, list(range(8)), trace=_trace)
    except Exception:
        if not _trace:
            raise
        res = run_bass_kernel_spmd(nc, in_maps, list(range(8)))
    kernel._last_run_seconds = _time.time() - t0
    bo = np.asarray(bo, np.float32)
    out = np.stack(
        [res.results[2 * b]["y"] + res.results[2 * b + 1]["y"] + bo
         for b in range(4)]
    ).astype(np.float32)
    if _trace:
        kernel._last_results = res
    return out
